# revision 2
# baseline (speedup 1.0000x reference)
"""DeepSeek-V3 MLA forward (B=1, S=2048, D=4096, H=32) on 8 TRN2 NeuronCores.

v2: tensor-parallel over heads (4 heads/core) for the b-projections /
attention / out-proj, sequence-parallel for the low-rank a-projections.
Each core computes the a-projections (q_a, compressed kv, shared rope key)
for 4 strips of 64 columns (one strip per 512-wide query tile), normalizes
them locally (rms scales, ln weights folded into the b-weights host-side),
and the strips are exchanged with device AllGathers: one early gather for
ckv+krope (feeds the kv b-projection), then one qa gather per query tile
issued in reverse tile order so attention for the largest (last) causal
block starts as early as possible. The post-out-proj all-reduce stays
host-side (sum of per-core partials).

Layouts follow v1: activations feature-major (x^T), scores transposed so
softmax reduction runs on the PE via ones-matmuls, P@V consumes exp^T
directly. All matmul operands bf16 (f32r for rope rotation / broadcast
helpers), accumulation fp32 in PSUM.
"""

import math
from dataclasses import dataclass

import ml_dtypes
import numpy as np

import concourse.bass as bass
import concourse.mybir as mybir
import concourse.tile as tile
from concourse import bacc
from concourse.bass_utils import run_bass_kernel_spmd

F32 = mybir.dt.float32
F32R = mybir.dt.float32r
BF16 = mybir.dt.bfloat16
AF = mybir.ActivationFunctionType
BF16NP = ml_dtypes.bfloat16

N_CORES = 8
EPS = 1e-6
THETA = 10000.0


@dataclass(frozen=True)
class Cfg:
    S: int = 2048
    D: int = 4096
    QR: int = 1536      # q lora rank
    KVR: int = 512      # kv lora rank
    H: int = 32         # total heads
    HPC: int = 4        # heads per core
    NOPE: int = 128
    ROPE: int = 64
    VD: int = 128
    STRIP: int = 64     # per-core columns per query tile

    @property
    def QD(self):
        return self.NOPE + self.ROPE

    @property
    def DCH(self):
        return self.D // 128

    @property
    def QRCH(self):
        return self.QR // 128

    @property
    def KVCH(self):
        return self.KVR // 128

    @property
    def AM(self):          # a-proj m-chunks: q rank + kv rank + 1 rope(64pad)
        return self.QRCH + self.KVCH + 1

    @property
    def NQT(self):         # 512-wide query tiles
        return self.S // 512

    @property
    def NKI(self):         # 128-wide key blocks
        return self.S // 128

    @property
    def LC(self):          # local a-proj columns (one strip per query tile)
        return self.NQT * self.STRIP


FULL = Cfg()


# --------------------------------------------------------------------------
# host-side input preparation
# --------------------------------------------------------------------------

def _rope_perm(rope):
    # deepseek interleave: xp = concat(x[0::2], x[1::2]) acting on rope dims
    return np.concatenate([np.arange(0, rope, 2), np.arange(1, rope, 2)])


def prep_inputs(cfg, hidden_states, Wq_a, q_a_ln_w, Wq_b, Wkv_a, kv_a_ln_w,
                Wkv_b, Wo):
    c = cfg
    hs = np.asarray(hidden_states, np.float32).reshape(c.S, c.D)
    Wq_a = np.asarray(Wq_a, np.float32)
    Wq_b = np.asarray(Wq_b, np.float32)
    Wkv_a = np.asarray(Wkv_a, np.float32)
    Wkv_b = np.asarray(Wkv_b, np.float32)
    Wo = np.asarray(Wo, np.float32)
    q_a_ln_w = np.asarray(q_a_ln_w, np.float32)
    kv_a_ln_w = np.asarray(kv_a_ln_w, np.float32)

    hT = np.ascontiguousarray(hs.T)                      # [D, S]

    # combined a-proj weight, padded to AM*128 cols, ckv chunks FIRST so the
    # ckv+krope gather can start before the (3x larger) q chunks compute.
    # Layout [AM, 128, DCH*128] so each m-chunk loads with one contiguous DMA.
    perm_a = _rope_perm(c.ROPE)
    wa = np.concatenate(
        [Wkv_a[:, :c.KVR],                               # ckv chunks 0..3
         Wkv_a[:, c.KVR:][:, perm_a],                    # rope (64)
         np.zeros((c.D, 128 - c.ROPE), np.float32),      # pad rope chunk
         Wq_a], axis=1)                                  # [D, AM*128]
    wa = wa.reshape(c.DCH, 128, c.AM, 128)               # [dch, p, m, c]
    wa = np.ascontiguousarray(wa.transpose(2, 1, 0, 3))  # [m, p, dch, c]
    wa = wa.reshape(c.AM, 128, c.D)

    # per-head-group b-projections / out-proj
    qd, nope, rope, vd = c.QD, c.NOPE, c.ROPE, c.VD
    scale = qd ** (-0.5)
    wqb_all = (Wq_b * q_a_ln_w[:, None]).reshape(c.QR, c.H, qd) * scale
    perm = _rope_perm(rope)
    wqb_nope = wqb_all[:, :, :nope]
    wqb_rope = wqb_all[:, :, nope:][:, :, perm]
    wkv_all = (Wkv_b * kv_a_ln_w[:, None]).reshape(c.KVR, c.H, nope + vd)

    # rotary tables, feature-major, replicated to 128 rows
    inv_freq = 1.0 / (THETA ** (np.arange(0, rope, 2, np.float32) / rope))
    freqs = np.outer(np.arange(c.S, dtype=np.float32), inv_freq)  # [S, 32]
    cosT = np.tile(np.cos(freqs).T, (4, 1)).astype(np.float32)    # [128, S]
    sinT = np.tile(np.sin(freqs).T, (4, 1)).astype(np.float32)
    # rotate-half as a PE matmul: rot = R @ x, R block-diag over two 64-row
    # rope groups, R = [[0, -I32], [I32, 0]] per group. lhsT = R.T.
    R = np.zeros((128, 128), np.float32)
    for blk in (0, 64):
        for i in range(32):
            R[blk + i, blk + i + 32] = -1.0
            R[blk + i + 32, blk + i] = 1.0
    rotT = np.ascontiguousarray(R.T)

    # diagonal-tile masks: mask01[j][r, q] = 1 if 128*j + r <= q
    j = np.arange(4)[:, None, None]
    r = np.arange(128)[None, :, None]
    q = np.arange(512)[None, None, :]
    mask01 = ((128 * j + r) <= q).astype(BF16NP)
    mask01 = np.ascontiguousarray(
        mask01.transpose(1, 0, 2)).reshape(128, 4 * 512)

    shared = {
        "wa": wa.astype(BF16NP),
        "cosT": cosT.astype(BF16NP),
        "sinT": sinT.astype(BF16NP),
        "rotT": rotT,
        "ones_f": np.ones((128, 128), np.float32),
        "mask01": mask01,
    }
    in_maps = []
    for core in range(N_CORES):
        # local columns: strip `core` of each query tile
        cols = np.concatenate(
            [512 * qi + 64 * core + np.arange(64) for qi in range(c.NQT)])
        # [128, DCH*LC] so the whole local slice loads as one contiguous DMA
        hT_loc = np.ascontiguousarray(
            hT[:, cols].reshape(c.DCH, 128, c.LC).transpose(1, 0, 2)
        ).reshape(128, c.DCH * c.LC)
        cos_loc = cosT[0:64, cols]
        sin_loc = sinT[0:64, cols]

        hsel = np.arange(core * c.HPC, (core + 1) * c.HPC)
        wqb_c = np.concatenate(
            [wqb_nope[:, hsel].reshape(c.QR, c.HPC * nope),
             wqb_rope[:, hsel].reshape(c.QR, c.HPC * rope)], axis=1)
        wqb_c = np.ascontiguousarray(
            wqb_c.reshape(c.QRCH, 128, c.HPC * qd)).astype(BF16NP)
        wkb_c = np.ascontiguousarray(
            wkv_all[:, hsel, :nope].reshape(c.KVCH, 128, c.HPC * nope)
        ).astype(BF16NP)
        wv_c = np.ascontiguousarray(
            wkv_all[:, hsel, nope:].reshape(c.KVCH, 128, c.HPC * vd)
        ).astype(BF16NP)
        wo_c = np.ascontiguousarray(
            Wo.reshape(c.H, vd, c.D)[hsel]).astype(BF16NP)
        in_maps.append(dict(
            shared,
            hT=hT_loc.astype(BF16NP),
            cos_loc=np.ascontiguousarray(cos_loc).astype(BF16NP),
            sin_loc=np.ascontiguousarray(sin_loc).astype(BF16NP),
            wqb=wqb_c, wkb=wkb_c, wv=wv_c, wo=wo_c))
    return in_maps


# --------------------------------------------------------------------------
# kernel builder
# --------------------------------------------------------------------------

def build(cfg):
    c = cfg
    nc = bacc.Bacc("TRN2", target_bir_lowering=False, debug=False,
                   num_devices=N_CORES)

    hT_d = nc.declare_dram_parameter("hT", [128, c.DCH * c.LC], BF16, isOutput=False)
    wa_d = nc.declare_dram_parameter("wa", [c.AM, 128, c.D], BF16, isOutput=False)
    wqb_d = nc.declare_dram_parameter("wqb", [c.QRCH, 128, c.HPC * c.QD], BF16, isOutput=False)
    wkb_d = nc.declare_dram_parameter("wkb", [c.KVCH, 128, c.HPC * c.NOPE], BF16, isOutput=False)
    wv_d = nc.declare_dram_parameter("wv", [c.KVCH, 128, c.HPC * c.VD], BF16, isOutput=False)
    wo_d = nc.declare_dram_parameter("wo", [c.HPC, 128, c.D], BF16, isOutput=False)
    cos_d = nc.declare_dram_parameter("cosT", [128, c.S], BF16, isOutput=False)
    sin_d = nc.declare_dram_parameter("sinT", [128, c.S], BF16, isOutput=False)
    cosl_d = nc.declare_dram_parameter("cos_loc", [64, c.LC], BF16, isOutput=False)
    sinl_d = nc.declare_dram_parameter("sin_loc", [64, c.LC], BF16, isOutput=False)
    rot_d = nc.declare_dram_parameter("rotT", [128, 128], F32R, isOutput=False)
    ones_d = nc.declare_dram_parameter("ones_f", [128, 128], F32R, isOutput=False)
    mask_d = nc.declare_dram_parameter("mask01", [128, 4 * 512], BF16, isOutput=False)
    out_d = nc.declare_dram_parameter("outT", [c.D, c.S], F32, isOutput=True)

    NQT = c.NQT
    CKCH = c.KVCH + 1                 # ckv chunks + krope chunk
    CKR = c.KVCH * 128 + 64           # gathered ckv rows (krope not padded)
    # gather bounce buffers (collectives need DRAM operands; out Shared)
    gin_ckv = nc.dram_tensor("gin_ckv", [NQT, CKR, 64], BF16)
    gout_ckv = nc.dram_tensor("gout_ckv", [N_CORES, NQT, CKR, 64], BF16,
                              addr_space="Shared")
    gin_qa = nc.dram_tensor("gin_qa", [NQT, c.QRCH, 128, 64], BF16)
    gout_qa = nc.dram_tensor("gout_qa", [NQT, N_CORES, c.QRCH, 128, 64], BF16,
                             addr_space="Shared")

    RG = [list(range(N_CORES))]
    NROPE_CH = c.HPC * c.ROPE // 128          # rope m-chunks in wqb (2)
    QB_M = c.HPC + NROPE_CH                   # 6

    with tile.TileContext(nc) as tc:
        with tc.tile_pool(name="persist", bufs=1) as pp:
            # persistent tiles
            cos_sb = pp.tile([128, c.S], BF16, name="cos_sb")
            sin_sb = pp.tile([128, c.S], BF16, name="sin_sb")
            rot_sb = pp.tile([128, 128], F32R, name="rot_sb")
            ones_sb = pp.tile([128, 128], F32R, name="ones_sb")
            nc.sync.dma_start(rot_sb[:], rot_d.ap())
            nc.sync.dma_start(ones_sb[:], ones_d.ap())
            ones_col_f = ones_sb[:, 0:1]
            ones_row_f = ones_sb[0:1, :]
            ones_col_b = pp.tile([128, 1], BF16, name="ones_col_b")
            ones_row_b = pp.tile([1, 128], BF16, name="ones_row_b")
            nc.vector.memset(ones_col_b[:], 1.0)
            nc.vector.memset(ones_row_b[:], 1.0)

            # B/C shared residents
            knopeT = [pp.tile([128, c.S], BF16, name=f"knopeT_{m}")
                      for m in range(c.HPC)]
            v_sb = [pp.tile([128, c.HPC * c.VD], BF16, name=f"v_sb_{ki}")
                    for ki in range(c.NKI)]
            krope2 = [pp.tile([128, c.S], BF16, name=f"krope2_{par}")
                      for par in range(2)]
            nc.vector.memset(krope2[0][:], 0.0)
            nc.vector.memset(krope2[1][:], 0.0)

            # ---------------- phase A: sharded a-projections -----------
            with tc.tile_pool(name="pA", bufs=1) as pA, \
                 tc.tile_pool(name="pA_w", bufs=3) as pAw, \
                 tc.tile_pool(name="pA_ev", bufs=4) as pAe, \
                 tc.tile_pool(name="pA_ps", bufs=2, space="PSUM") as psA, \
                 tc.tile_pool(name="pA_ps1", bufs=1, space="PSUM") as psA1:
                cosl_sb = pA.tile([64, c.LC], BF16, name="cosl_sb")
                sinl_sb = pA.tile([64, c.LC], BF16, name="sinl_sb")
                nc.sync.dma_start(cosl_sb[:], cosl_d.ap())
                nc.sync.dma_start(sinl_sb[:], sinl_d.ap())
                hT_all = pA.tile([128, c.DCH * c.LC], BF16, name="hT_all")
                nc.sync.dma_start(hT_all[:], hT_d.ap())
                hT_sb = [hT_all[:, k * c.LC:(k + 1) * c.LC]
                         for k in range(c.DCH)]

                ckv_all = pA.tile([128, c.KVCH * c.LC], BF16, name="ckv_all")
                ckv_ch = [ckv_all[:, i * c.LC:(i + 1) * c.LC]
                          for i in range(c.KVCH)]
                krope_ch = pA.tile([64, c.LC], BF16, name="krope_ch")
                qa_all = pA.tile([128, c.QRCH * c.LC], BF16, name="qa_all")
                qa_ch = [qa_all[:, i * c.LC:(i + 1) * c.LC]
                         for i in range(c.QRCH)]
                ssq = psA1.tile([1, c.LC], F32, name="ssq_q")
                ssc = psA1.tile([1, c.LC], F32, name="ssq_c")

                def a_norm(ps1, denom, chunks):
                    """rs = rsqrt(mean+eps) of ps1; chunks *= broadcast(rs)."""
                    t = pAe.tile([1, c.LC], F32, name="rms_t")
                    nc.vector.tensor_scalar(
                        t[:], ps1[:], 1.0 / denom, EPS,
                        mybir.AluOpType.mult, mybir.AluOpType.add)
                    st = pAe.tile([1, c.LC], F32, name="rms_st")
                    nc.scalar.activation(st[:], t[:], AF.Sqrt)
                    rc = pAe.tile([1, c.LC], F32R, name="rms_rc")
                    with nc.allow_low_precision(reason="fp32r for PE bcast"):
                        nc.vector.reciprocal(rc[:], st[:])
                    bc_ps = psA.tile([128, c.LC], F32, name="bc_ps", bufs=1)
                    nc.tensor.matmul(bc_ps[:], ones_row_f, rc[:])
                    bc_sb = pAe.tile([128, c.LC], F32, name="bc_sb")
                    nc.vector.tensor_copy(bc_sb[:], bc_ps[:])
                    for ch in chunks:
                        nc.vector.tensor_mul(ch, ch, bc_sb[:])

                # m order: ckv chunks, krope, then qa chunks
                for m in range(c.AM):
                    wa_sb = pAw.tile([128, c.D], BF16, name="wa_sb")
                    nc.sync.dma_start(wa_sb[:], wa_d.ap()[m])
                    ps = psA.tile([128, c.LC], F32, name="psA")
                    for k in range(c.DCH):
                        nc.tensor.matmul(
                            ps[:], wa_sb[:, k * 128:(k + 1) * 128],
                            hT_sb[k], start=(k == 0), stop=(k == c.DCH - 1))
                    if m < c.KVCH:
                        nc.vector.tensor_copy(ckv_ch[m], ps[:])
                        x2 = pAe.tile([128, c.LC], F32R, name="x2")
                        nc.vector.tensor_mul(x2[:], ckv_ch[m], ckv_ch[m])
                        nc.tensor.matmul(ssc[:], ones_col_f, x2[:],
                                         start=(m == 0), stop=(m == c.KVCH - 1))
                    elif m == c.KVCH:
                        # shared rope key: rows 0:64 of this chunk
                        kr = pAe.tile([64, c.LC], F32R, name="kr")
                        nc.vector.tensor_copy(kr[:], ps[0:64, :])
                        rps = psA.tile([64, c.LC], F32, name="rot_ps", bufs=1)
                        nc.tensor.matmul(rps[:], rot_sb[0:64, 0:64], kr[:])
                        rk = pAe.tile([64, c.LC], F32, name="rk")
                        nc.vector.tensor_copy(rk[:], rps[:])
                        a = pAe.tile([64, c.LC], F32, name="ra")
                        b = pAe.tile([64, c.LC], F32, name="rb")
                        nc.vector.tensor_mul(a[:], kr[:], cosl_sb[:])
                        nc.vector.tensor_mul(b[:], rk[:], sinl_sb[:])
                        nc.vector.tensor_add(krope_ch[:], a[:], b[:])
                    else:
                        mq = m - CKCH
                        nc.vector.tensor_copy(qa_ch[mq], ps[:])
                        x2 = pAe.tile([128, c.LC], F32R, name="x2")
                        nc.vector.tensor_mul(x2[:], qa_ch[mq], qa_ch[mq])
                        nc.tensor.matmul(ssq[:], ones_col_f, x2[:],
                                         start=(mq == 0),
                                         stop=(mq == c.QRCH - 1))
                    if m == c.KVCH - 1:
                        with tc.high_priority():
                            a_norm(ssc, c.KVR, ckv_ch)
                    if m == c.KVCH:
                        # ship ckv + krope strips, gather early
                        with tc.high_priority():
                            ckv_v = ckv_all[:].rearrange(
                                "p (k q x) -> p k q x", k=c.KVCH, q=NQT)
                            for qi in range(NQT):
                                nc.scalar.dma_start(
                                    gin_ckv.ap()[qi, 0:c.KVCH * 128]
                                    .rearrange("(k p) x -> p k x", k=c.KVCH),
                                    ckv_v[:, :, qi, :])
                                nc.scalar.dma_start(
                                    gin_ckv.ap()[qi, c.KVCH * 128:CKR],
                                    krope_ch[:, qi * 64:qi * 64 + 64])
                            nc.gpsimd.collective_compute(
                                "AllGather", mybir.AluOpType.bypass,
                                replica_groups=RG,
                                ins=[gin_ckv.ap().opt()],
                                outs=[gout_ckv.ap().opt()])
                # big rope tables are only needed by phase C q-rope
                nc.sync.dma_start(cos_sb[:], cos_d.ap())
                nc.sync.dma_start(sin_sb[:], sin_d.ap())
                with tc.high_priority():
                    a_norm(ssq, c.QR, qa_ch)
                    qa_v = qa_all[:].rearrange(
                        "p (k q x) -> p k q x", k=c.QRCH, q=NQT)
                    for qi in range(NQT - 1, -1, -1):
                        nc.scalar.dma_start(
                            gin_qa.ap()[qi].rearrange("k p x -> p k x"),
                            qa_v[:, :, qi, :])
                        nc.gpsimd.collective_compute(
                            "AllGather", mybir.AluOpType.bypass,
                            replica_groups=RG,
                            ins=[gin_qa.ap()[qi].opt()],
                            outs=[gout_qa.ap()[qi].opt()])

            # ---------------- phase B: kv b-projection -----------------
            with tc.tile_pool(name="pB", bufs=2) as pB, \
                 tc.tile_pool(name="pB_ev", bufs=4) as pBe, \
                 tc.tile_pool(name="pB_ps", bufs=3, space="PSUM") as psB:
                wkb_sb = []
                wv_sb = []
                for kc in range(c.KVCH):
                    t = pB.tile([128, c.HPC * c.NOPE], BF16, name=f"wkb_{kc}",
                                bufs=1)
                    nc.sync.dma_start(t[:], wkb_d.ap()[kc])
                    wkb_sb.append(t)
                    t = pB.tile([128, c.HPC * c.VD], BF16, name=f"wv_{kc}",
                                bufs=1)
                    nc.sync.dma_start(t[:], wv_d.ap()[kc])
                    wv_sb.append(t)
                # scheduler hint: B follows the ckv gather; keep it from
                # being interleaved before the phase-A norm/gather ops
                tc.tile_set_cur_wait(1)
                for j in range(NQT):
                    # gathered block j: cols [512j, 512j+512), 64-col runs
                    # interleaved across cores -> natural column order
                    ckv_T = []
                    for kc in range(c.KVCH):
                        t = pB.tile([128, 512], BF16, name=f"ckvT_{kc}")
                        nc.sync.dma_start(
                            t[:].rearrange("p (c x) -> p c x", c=N_CORES),
                            gout_ckv.ap()[:, j, kc * 128:(kc + 1) * 128]
                            .rearrange("c p x -> p c x"))
                        ckv_T.append(t)
                    for par in range(2):
                        nc.sync.dma_start(
                            krope2[par][64 * par:64 * par + 64,
                                        512 * j:512 * (j + 1)].rearrange(
                                "p (c x) -> p c x", c=N_CORES),
                            gout_ckv.ap()[:, j, c.KVCH * 128:CKR]
                            .rearrange("c p x -> p c x"))
                    for m in range(c.HPC):
                        ps = psB.tile([128, 512], F32, name="psB")
                        for kc in range(c.KVCH):
                            nc.tensor.matmul(
                                ps[:], wkb_sb[kc][:, m * 128:(m + 1) * 128],
                                ckv_T[kc][:], start=(kc == 0),
                                stop=(kc == c.KVCH - 1))
                        nc.vector.tensor_copy(
                            knopeT[m][:, 512 * j:512 * (j + 1)], ps[:])
                    for kk in range(4):
                        ki = 4 * j + kk
                        ps = psB.tile([128, c.HPC * c.VD], F32, name="psB")
                        for kc in range(c.KVCH):
                            nc.tensor.matmul(
                                ps[:], ckv_T[kc][:, kk * 128:(kk + 1) * 128],
                                wv_sb[kc][:], start=(kc == 0),
                                stop=(kc == c.KVCH - 1))
                        nc.vector.tensor_copy(v_sb[ki][:], ps[:])

            # ---------------- phase C: q, attention, out-proj ----------
            with tc.tile_pool(name="pC", bufs=1) as pC, \
                 tc.tile_pool(name="pC2", bufs=2) as pC2, \
                 tc.tile_pool(name="pCe", bufs=2) as pCe, \
                 tc.tile_pool(name="pCo", bufs=6) as pCo, \
                 tc.tile_pool(name="pCx", bufs=6) as pCx, \
                 tc.tile_pool(name="pC_mm", bufs=2, space="PSUM") as psM, \
                 tc.tile_pool(name="pC_sT", bufs=3, space="PSUM") as psT, \
                 tc.tile_pool(name="pC_oT", bufs=2, space="PSUM") as psO, \
                 tc.tile_pool(name="pC_den", bufs=1, space="PSUM") as psD:
                wqb_sb = []
                for k in range(c.QRCH):
                    t = pC.tile([128, c.HPC * c.QD], BF16, name=f"wqb_{k}")
                    nc.sync.dma_start(t[:], wqb_d.ap()[k])
                    wqb_sb.append(t)
                wo_sb = []
                for k in range(c.HPC):
                    t = pC.tile([128, c.D], BF16, name=f"wo_{k}")
                    nc.sync.dma_start(t[:], wo_d.ap()[k])
                    wo_sb.append(t)
                mask_all = pC.tile([128, 4 * 512], BF16, name="mask_all")
                nc.sync.dma_start(mask_all[:], mask_d.ap())
                mask_sb = [mask_all[:, j * 512:(j + 1) * 512]
                           for j in range(4)]

                for qi in range(NQT - 1, -1, -1):
                    # scheduler hint: C tiles run in reverse qi order after B
                    tc.tile_set_cur_wait(2 + (NQT - 1 - qi))
                    q0 = qi * 512
                    qa_sb = []
                    for k in range(c.QRCH):
                        t = pC2.tile([128, 512], BF16, name=f"qa_{k}")
                        nc.sync.dma_start(
                            t[:].rearrange("p (c x) -> p c x", c=N_CORES),
                            gout_qa.ap()[qi, :, k].rearrange("c p x -> p c x"))
                        qa_sb.append(t)

                    qnopeT = [pC2.tile([128, 512], BF16, name=f"qnopeT_{m}")
                              for m in range(c.HPC)]
                    qrope_ch = [pC2.tile([128, 512], BF16, name=f"qrope_{j}")
                                for j in range(NROPE_CH)]
                    for m in range(QB_M):
                        ps = psM.tile([128, 512], F32, name="psm")
                        for k in range(c.QRCH):
                            nc.tensor.matmul(
                                ps[:], wqb_sb[k][:, m * 128:(m + 1) * 128],
                                qa_sb[k][:], start=(k == 0),
                                stop=(k == c.QRCH - 1))
                        if m < c.HPC:
                            nc.vector.tensor_copy(qnopeT[m][:], ps[:])
                        else:
                            ro = pCe.tile([128, 512], F32R, name="ro")
                            nc.vector.tensor_copy(ro[:], ps[:])
                            rps = psM.tile([128, 512], F32, name="psm")
                            nc.tensor.matmul(rps[:], rot_sb[:], ro[:])
                            rk = pCe.tile([128, 512], F32, name="qrk")
                            nc.vector.tensor_copy(rk[:], rps[:])
                            a = pCe.tile([128, 512], F32, name="qra")
                            b = pCe.tile([128, 512], F32, name="qrb")
                            nc.vector.tensor_mul(
                                a[:], ro[:], cos_sb[:, q0:q0 + 512])
                            nc.vector.tensor_mul(
                                b[:], rk[:], sin_sb[:, q0:q0 + 512])
                            nc.vector.tensor_add(qrope_ch[m - c.HPC][:],
                                                 a[:], b[:])

                    oT_sb = [pC2.tile([128, 512], BF16, name=f"oT_{h}")
                             for h in range(c.HPC)]
                    nki = 4 * (qi + 1)
                    for h in range(c.HPC):
                        oT_ps = psO.tile([128, 512], F32, name="psO")
                        den_ps = psD.tile([1, 512], F32, name="psD")
                        for ki in range(nki):
                            sT_ps = psT.tile([128, 512], F32, name="psT")
                            nc.tensor.matmul(
                                sT_ps[:],
                                knopeT[h][:, ki * 128:(ki + 1) * 128],
                                qnopeT[h][:], start=True, stop=False)
                            nc.tensor.matmul(
                                sT_ps[:],
                                krope2[h % 2][:, ki * 128:(ki + 1) * 128],
                                qrope_ch[h // 2][:], start=False, stop=True)
                            ex = pCx.tile([128, 512], BF16, name="expT")
                            nc.scalar.activation(ex[:], sT_ps[:], AF.Exp)
                            jj = ki - (nki - 4)
                            if jj >= 0:
                                nc.vector.tensor_mul(ex[:], ex[:],
                                                     mask_sb[jj])
                            nc.tensor.matmul(den_ps[:], ones_col_b[:], ex[:],
                                             start=(ki == 0),
                                             stop=(ki == nki - 1))
                            nc.tensor.matmul(
                                oT_ps[:], v_sb[ki][:, h * c.VD:(h + 1) * c.VD],
                                ex[:], start=(ki == 0), stop=(ki == nki - 1))
                        rec = pCe.tile([1, 512], F32R, name="rec")
                        with nc.allow_low_precision(reason="fp32r for PE bcast"):
                            nc.vector.reciprocal(rec[:], den_ps[:])
                        bc_ps = psM.tile([128, 512], F32, name="psm")
                        nc.tensor.matmul(bc_ps[:], ones_row_f, rec[:])
                        bc_sb = pCe.tile([128, 512], F32, name="bc_sb")
                        nc.vector.tensor_copy(bc_sb[:], bc_ps[:])
                        nc.vector.tensor_mul(oT_sb[h][:], oT_ps[:], bc_sb[:])

                    for m in range(c.DCH):
                        ps = psM.tile([128, 512], F32, name="psm")
                        for k in range(c.HPC):
                            nc.tensor.matmul(
                                ps[:], wo_sb[k][:, m * 128:(m + 1) * 128],
                                oT_sb[k][:], start=(k == 0),
                                stop=(k == c.HPC - 1))
                        ob = pCo.tile([128, 512], F32, name="ob")
                        nc.vector.tensor_copy(ob[:], ps[:])
                        nc.scalar.dma_start(
                            out_d.ap()[m * 128:(m + 1) * 128, q0:q0 + 512],
                            ob[:])
    nc.compile()
    return nc


# --------------------------------------------------------------------------
# public entry point
# --------------------------------------------------------------------------

_CACHED = {}


def _get_nc(cfg):
    key = cfg
    if key not in _CACHED:
        _CACHED[key] = build(cfg)
    return _CACHED[key]


def kernel(hidden_states, Wq_a, q_a_ln_w, Wq_b, Wkv_a, kv_a_ln_w, Wkv_b, Wo):
    cfg = FULL
    in_maps = prep_inputs(cfg, hidden_states, Wq_a, q_a_ln_w, Wq_b, Wkv_a,
                          kv_a_ln_w, Wkv_b, Wo)
    nc = _get_nc(cfg)
    res = run_bass_kernel_spmd(nc, in_maps, core_ids=list(range(N_CORES)))
    acc = np.zeros((cfg.D, cfg.S), np.float32)
    for r in res.results:
        acc += r["outT"]
    return np.ascontiguousarray(acc.T).reshape(1, cfg.S, cfg.D)


# revision 3
# speedup vs baseline: 1.0146x; 1.0146x over previous
"""DeepSeek-V3 MLA forward (B=1, S=2048, D=4096, H=32) on 8 TRN2 NeuronCores.

v2: tensor-parallel over heads (4 heads/core) for the b-projections /
attention / out-proj, sequence-parallel for the low-rank a-projections.
Each core computes the a-projections (q_a, compressed kv, shared rope key)
for 4 strips of 64 columns (one strip per 512-wide query tile), normalizes
them locally (rms scales, ln weights folded into the b-weights host-side),
and the strips are exchanged with device AllGathers: one early gather for
ckv+krope (feeds the kv b-projection), then one qa gather per query tile
issued in reverse tile order so attention for the largest (last) causal
block starts as early as possible. The post-out-proj all-reduce stays
host-side (sum of per-core partials).

Layouts follow v1: activations feature-major (x^T), scores transposed so
softmax reduction runs on the PE via ones-matmuls, P@V consumes exp^T
directly. All matmul operands bf16 (f32r for rope rotation / broadcast
helpers), accumulation fp32 in PSUM.
"""

import math
from dataclasses import dataclass

import ml_dtypes
import numpy as np

import concourse.bass as bass
import concourse.bass_isa as bass_isa
import concourse.mybir as mybir
import concourse.tile as tile
from concourse import bacc
from concourse.bass_utils import run_bass_kernel_spmd

F32 = mybir.dt.float32
F32R = mybir.dt.float32r
BF16 = mybir.dt.bfloat16
AF = mybir.ActivationFunctionType
BF16NP = ml_dtypes.bfloat16

N_CORES = 8
EPS = 1e-6
THETA = 10000.0


@dataclass(frozen=True)
class Cfg:
    S: int = 2048
    D: int = 4096
    QR: int = 1536      # q lora rank
    KVR: int = 512      # kv lora rank
    H: int = 32         # total heads
    HPC: int = 4        # heads per core
    NOPE: int = 128
    ROPE: int = 64
    VD: int = 128
    STRIP: int = 64     # per-core columns per query tile

    @property
    def QD(self):
        return self.NOPE + self.ROPE

    @property
    def DCH(self):
        return self.D // 128

    @property
    def QRCH(self):
        return self.QR // 128

    @property
    def KVCH(self):
        return self.KVR // 128

    @property
    def AM(self):          # a-proj m-chunks: q rank + kv rank + 1 rope(64pad)
        return self.QRCH + self.KVCH + 1

    @property
    def NQT(self):         # 512-wide query tiles
        return self.S // 512

    @property
    def NKI(self):         # 128-wide key blocks
        return self.S // 128

    @property
    def LC(self):          # local a-proj columns (one strip per query tile)
        return self.NQT * self.STRIP


FULL = Cfg()


# --------------------------------------------------------------------------
# host-side input preparation
# --------------------------------------------------------------------------

def _rope_perm(rope):
    # deepseek interleave: xp = concat(x[0::2], x[1::2]) acting on rope dims
    return np.concatenate([np.arange(0, rope, 2), np.arange(1, rope, 2)])


def prep_inputs(cfg, hidden_states, Wq_a, q_a_ln_w, Wq_b, Wkv_a, kv_a_ln_w,
                Wkv_b, Wo):
    c = cfg
    hs = np.asarray(hidden_states, np.float32).reshape(c.S, c.D)
    Wq_a = np.asarray(Wq_a, np.float32)
    Wq_b = np.asarray(Wq_b, np.float32)
    Wkv_a = np.asarray(Wkv_a, np.float32)
    Wkv_b = np.asarray(Wkv_b, np.float32)
    Wo = np.asarray(Wo, np.float32)
    q_a_ln_w = np.asarray(q_a_ln_w, np.float32)
    kv_a_ln_w = np.asarray(kv_a_ln_w, np.float32)

    hT = np.ascontiguousarray(hs.T)                      # [D, S]

    # combined a-proj weight, padded to AM*128 cols, ckv chunks FIRST so the
    # ckv+krope gather can start before the (3x larger) q chunks compute.
    # Layout [AM, 128, DCH*128] so each m-chunk loads with one contiguous DMA.
    perm_a = _rope_perm(c.ROPE)
    wa = np.concatenate(
        [Wkv_a[:, :c.KVR],                               # ckv chunks 0..3
         Wkv_a[:, c.KVR:][:, perm_a],                    # rope (64)
         np.zeros((c.D, 128 - c.ROPE), np.float32),      # pad rope chunk
         Wq_a], axis=1)                                  # [D, AM*128]
    wa = wa.reshape(c.DCH, 128, c.AM, 128)               # [dch, p, m, c]
    wa = np.ascontiguousarray(wa.transpose(2, 1, 0, 3))  # [m, p, dch, c]
    wa = wa.reshape(c.AM, 128, c.D)

    # per-head-group b-projections / out-proj
    qd, nope, rope, vd = c.QD, c.NOPE, c.ROPE, c.VD
    scale = qd ** (-0.5)
    wqb_all = (Wq_b * q_a_ln_w[:, None]).reshape(c.QR, c.H, qd) * scale
    perm = _rope_perm(rope)
    wqb_nope = wqb_all[:, :, :nope]
    wqb_rope = wqb_all[:, :, nope:][:, :, perm]
    wkv_all = (Wkv_b * kv_a_ln_w[:, None]).reshape(c.KVR, c.H, nope + vd)

    # rotary tables, feature-major, replicated to 128 rows
    inv_freq = 1.0 / (THETA ** (np.arange(0, rope, 2, np.float32) / rope))
    freqs = np.outer(np.arange(c.S, dtype=np.float32), inv_freq)  # [S, 32]
    cosT = np.tile(np.cos(freqs).T, (4, 1)).astype(np.float32)    # [128, S]
    sinT = np.tile(np.sin(freqs).T, (4, 1)).astype(np.float32)
    # rotate-half as a PE matmul: rot = R @ x, R block-diag over two 64-row
    # rope groups, R = [[0, -I32], [I32, 0]] per group. lhsT = R.T.
    R = np.zeros((128, 128), np.float32)
    for blk in (0, 64):
        for i in range(32):
            R[blk + i, blk + i + 32] = -1.0
            R[blk + i + 32, blk + i] = 1.0
    rotT = np.ascontiguousarray(R.T)

    # diagonal-tile masks: mask01[j][r, q] = 1 if 128*j + r <= q
    j = np.arange(4)[:, None, None]
    r = np.arange(128)[None, :, None]
    q = np.arange(512)[None, None, :]
    mask01 = ((128 * j + r) <= q).astype(BF16NP)
    mask01 = np.ascontiguousarray(
        mask01.transpose(1, 0, 2)).reshape(128, 4 * 512)

    shared = {
        "wa": wa.astype(BF16NP),
        "cosT": cosT.astype(BF16NP),
        "sinT": sinT.astype(BF16NP),
        "rotT": rotT,
        "ones_f": np.ones((128, 128), np.float32),
        "mask01": mask01,
    }
    in_maps = []
    for core in range(N_CORES):
        # local columns: strip `core` of each query tile
        cols = np.concatenate(
            [512 * qi + 64 * core + np.arange(64) for qi in range(c.NQT)])
        # [128, DCH*LC] so the whole local slice loads as one contiguous DMA
        hT_loc = np.ascontiguousarray(
            hT[:, cols].reshape(c.DCH, 128, c.LC).transpose(1, 0, 2)
        ).reshape(128, c.DCH * c.LC)
        cos_loc = cosT[0:64, cols]
        sin_loc = sinT[0:64, cols]

        hsel = np.arange(core * c.HPC, (core + 1) * c.HPC)
        wqb_c = np.concatenate(
            [wqb_nope[:, hsel].reshape(c.QR, c.HPC * nope),
             wqb_rope[:, hsel].reshape(c.QR, c.HPC * rope)], axis=1)
        wqb_c = np.ascontiguousarray(
            wqb_c.reshape(c.QRCH, 128, c.HPC * qd)).astype(BF16NP)
        wkb_c = np.ascontiguousarray(
            wkv_all[:, hsel, :nope].reshape(c.KVCH, 128, c.HPC * nope)
        ).astype(BF16NP)
        wv_c = np.ascontiguousarray(
            wkv_all[:, hsel, nope:].reshape(c.KVCH, 128, c.HPC * vd)
        ).astype(BF16NP)
        wo_c = np.ascontiguousarray(
            Wo.reshape(c.H, vd, c.D)[hsel]).astype(BF16NP)
        in_maps.append(dict(
            shared,
            hT=hT_loc.astype(BF16NP),
            cos_loc=np.ascontiguousarray(cos_loc).astype(BF16NP),
            sin_loc=np.ascontiguousarray(sin_loc).astype(BF16NP),
            wqb=wqb_c, wkb=wkb_c, wv=wv_c, wo=wo_c))
    return in_maps


# --------------------------------------------------------------------------
# kernel builder
# --------------------------------------------------------------------------

def build(cfg):
    c = cfg
    nc = bacc.Bacc("TRN2", target_bir_lowering=False, debug=False,
                   num_devices=N_CORES)

    hT_d = nc.declare_dram_parameter("hT", [128, c.DCH * c.LC], BF16, isOutput=False)
    wa_d = nc.declare_dram_parameter("wa", [c.AM, 128, c.D], BF16, isOutput=False)
    wqb_d = nc.declare_dram_parameter("wqb", [c.QRCH, 128, c.HPC * c.QD], BF16, isOutput=False)
    wkb_d = nc.declare_dram_parameter("wkb", [c.KVCH, 128, c.HPC * c.NOPE], BF16, isOutput=False)
    wv_d = nc.declare_dram_parameter("wv", [c.KVCH, 128, c.HPC * c.VD], BF16, isOutput=False)
    wo_d = nc.declare_dram_parameter("wo", [c.HPC, 128, c.D], BF16, isOutput=False)
    cos_d = nc.declare_dram_parameter("cosT", [128, c.S], BF16, isOutput=False)
    sin_d = nc.declare_dram_parameter("sinT", [128, c.S], BF16, isOutput=False)
    cosl_d = nc.declare_dram_parameter("cos_loc", [64, c.LC], BF16, isOutput=False)
    sinl_d = nc.declare_dram_parameter("sin_loc", [64, c.LC], BF16, isOutput=False)
    rot_d = nc.declare_dram_parameter("rotT", [128, 128], F32R, isOutput=False)
    ones_d = nc.declare_dram_parameter("ones_f", [128, 128], F32R, isOutput=False)
    mask_d = nc.declare_dram_parameter("mask01", [128, 4 * 512], BF16, isOutput=False)
    out_d = nc.declare_dram_parameter("outT", [c.D, c.S], F32, isOutput=True)

    NQT = c.NQT
    CKCH = c.KVCH + 1                 # ckv chunks + krope chunk
    CKR = c.KVCH * 128 + 64           # gathered ckv rows (krope not padded)
    # gather bounce buffers (collectives need DRAM operands; out Shared)
    gin_ckv = nc.dram_tensor("gin_ckv", [NQT, CKR, 64], BF16)
    gout_ckv = nc.dram_tensor("gout_ckv", [N_CORES, NQT, CKR, 64], BF16,
                              addr_space="Shared")
    gin_qa = nc.dram_tensor("gin_qa", [NQT, c.QRCH, 128, 64], BF16)
    gout_qa = nc.dram_tensor("gout_qa", [NQT, N_CORES, c.QRCH, 128, 64], BF16,
                             addr_space="Shared")

    RG = [list(range(N_CORES))]
    NROPE_CH = c.HPC * c.ROPE // 128          # rope m-chunks in wqb (2)
    QB_M = c.HPC + NROPE_CH                   # 6

    with tile.TileContext(nc) as tc:
        with tc.tile_pool(name="persist", bufs=1) as pp:
            # persistent tiles
            cos_sb = pp.tile([128, c.S], BF16, name="cos_sb")
            sin_sb = pp.tile([128, c.S], BF16, name="sin_sb")
            rot_sb = pp.tile([128, 128], F32R, name="rot_sb")
            ones_sb = pp.tile([128, 128], F32R, name="ones_sb")
            nc.sync.dma_start(rot_sb[:], rot_d.ap())
            nc.sync.dma_start(ones_sb[:], ones_d.ap())
            ones_col_f = ones_sb[:, 0:1]
            ones_row_f = ones_sb[0:1, :]
            ones_col_b = pp.tile([128, 1], BF16, name="ones_col_b")
            ones_row_b = pp.tile([1, 128], BF16, name="ones_row_b")
            nc.vector.memset(ones_col_b[:], 1.0)
            nc.vector.memset(ones_row_b[:], 1.0)

            # B/C shared residents
            knopeT = [pp.tile([128, c.S], BF16, name=f"knopeT_{m}")
                      for m in range(c.HPC)]
            v_sb = [pp.tile([128, c.HPC * c.VD], BF16, name=f"v_sb_{ki}")
                    for ki in range(c.NKI)]
            krope2 = [pp.tile([128, c.S], BF16, name=f"krope2_{par}")
                      for par in range(2)]
            nc.vector.memset(krope2[0][:], 0.0)
            nc.vector.memset(krope2[1][:], 0.0)

            # ---------------- phase A: sharded a-projections -----------
            with tc.tile_pool(name="pA", bufs=1) as pA, \
                 tc.tile_pool(name="pA_w", bufs=3) as pAw, \
                 tc.tile_pool(name="pA_ev", bufs=4) as pAe, \
                 tc.tile_pool(name="pA_ps", bufs=2, space="PSUM") as psA, \
                 tc.tile_pool(name="pA_ps1", bufs=1, space="PSUM") as psA1:
                # first two a-proj weight chunks load before anything else so
                # the PE can start as soon as the first hT chunk lands
                wa_pre = []
                for m in range(2):
                    t = pAw.tile([128, c.D], BF16, name="wa_sb")
                    nc.sync.dma_start(t[:], wa_d.ap()[m])
                    wa_pre.append(t)
                cosl_sb = pA.tile([64, c.LC], BF16, name="cosl_sb")
                sinl_sb = pA.tile([64, c.LC], BF16, name="sinl_sb")
                nc.sync.dma_start(cosl_sb[:], cosl_d.ap())
                nc.sync.dma_start(sinl_sb[:], sinl_d.ap())
                hT_all = pA.tile([128, c.DCH * c.LC], BF16, name="hT_all")
                for q in range(4):
                    w = c.DCH * c.LC // 4
                    nc.sync.dma_start(hT_all[:, q * w:(q + 1) * w],
                                      hT_d.ap()[:, q * w:(q + 1) * w])
                hT_sb = [hT_all[:, k * c.LC:(k + 1) * c.LC]
                         for k in range(c.DCH)]

                ckv_all = pA.tile([128, c.KVCH * c.LC], BF16, name="ckv_all")
                ckv_ch = [ckv_all[:, i * c.LC:(i + 1) * c.LC]
                          for i in range(c.KVCH)]
                krope_ch = pA.tile([64, c.LC], BF16, name="krope_ch")
                qa_all = pA.tile([128, c.QRCH * c.LC], BF16, name="qa_all")
                qa_ch = [qa_all[:, i * c.LC:(i + 1) * c.LC]
                         for i in range(c.QRCH)]
                ssq = psA1.tile([1, c.LC], F32, name="ssq_q")
                ssc = psA1.tile([1, c.LC], F32, name="ssq_c")

                def a_norm(ps1, denom, chunks):
                    """rs = rsqrt(mean+eps) of ps1; chunks *= broadcast(rs)."""
                    t = pAe.tile([1, c.LC], F32, name="rms_t")
                    nc.vector.tensor_scalar(
                        t[:], ps1[:], 1.0 / denom, EPS,
                        mybir.AluOpType.mult, mybir.AluOpType.add)
                    st = pAe.tile([1, c.LC], F32, name="rms_st")
                    nc.scalar.activation(st[:], t[:], AF.Sqrt)
                    rc = pAe.tile([1, c.LC], F32R, name="rms_rc")
                    with nc.allow_low_precision(reason="fp32r for PE bcast"):
                        nc.vector.reciprocal(rc[:], st[:])
                    bc_ps = psA.tile([128, c.LC], F32, name="bc_ps", bufs=1)
                    nc.tensor.matmul(bc_ps[:], ones_row_f, rc[:])
                    bc_sb = pAe.tile([128, c.LC], F32, name="bc_sb")
                    nc.vector.tensor_copy(bc_sb[:], bc_ps[:])
                    for ch in chunks:
                        nc.vector.tensor_mul(ch, ch, bc_sb[:])

                # m order: ckv chunks, krope, then qa chunks
                for m in range(c.AM):
                    if m < 2:
                        wa_sb = wa_pre[m]
                    else:
                        wa_sb = pAw.tile([128, c.D], BF16, name="wa_sb")
                        nc.sync.dma_start(wa_sb[:], wa_d.ap()[m])
                    ps = psA.tile([128, c.LC], F32, name="psA")
                    for k in range(c.DCH):
                        nc.tensor.matmul(
                            ps[:], wa_sb[:, k * 128:(k + 1) * 128],
                            hT_sb[k], start=(k == 0), stop=(k == c.DCH - 1))
                    if m < c.KVCH:
                        nc.vector.tensor_copy(ckv_ch[m], ps[:])
                        x2 = pAe.tile([128, c.LC], F32R, name="x2")
                        nc.vector.tensor_mul(x2[:], ckv_ch[m], ckv_ch[m])
                        nc.tensor.matmul(ssc[:], ones_col_f, x2[:],
                                         start=(m == 0), stop=(m == c.KVCH - 1))
                    elif m == c.KVCH:
                        # shared rope key: rows 0:64 of this chunk
                        kr = pAe.tile([64, c.LC], F32R, name="kr")
                        nc.vector.tensor_copy(kr[:], ps[0:64, :])
                        rps = psA.tile([64, c.LC], F32, name="rot_ps", bufs=1)
                        nc.tensor.matmul(rps[:], rot_sb[0:64, 0:64], kr[:])
                        rk = pAe.tile([64, c.LC], F32, name="rk")
                        nc.vector.tensor_copy(rk[:], rps[:])
                        a = pAe.tile([64, c.LC], F32, name="ra")
                        b = pAe.tile([64, c.LC], F32, name="rb")
                        nc.vector.tensor_mul(a[:], kr[:], cosl_sb[:])
                        nc.vector.tensor_mul(b[:], rk[:], sinl_sb[:])
                        nc.vector.tensor_add(krope_ch[:], a[:], b[:])
                    else:
                        mq = m - CKCH
                        nc.vector.tensor_copy(qa_ch[mq], ps[:])
                        x2 = pAe.tile([128, c.LC], F32R, name="x2")
                        nc.vector.tensor_mul(x2[:], qa_ch[mq], qa_ch[mq])
                        nc.tensor.matmul(ssq[:], ones_col_f, x2[:],
                                         start=(mq == 0),
                                         stop=(mq == c.QRCH - 1))
                    if m == c.KVCH - 1:
                        with tc.high_priority():
                            a_norm(ssc, c.KVR, ckv_ch)
                    if m == c.KVCH:
                        # ship ckv + krope strips, gather early
                        with tc.high_priority():
                            ckv_v = ckv_all[:].rearrange(
                                "p (k q x) -> p k q x", k=c.KVCH, q=NQT)
                            for qi in range(NQT):
                                nc.scalar.dma_start(
                                    gin_ckv.ap()[qi, 0:c.KVCH * 128]
                                    .rearrange("(k p) x -> p k x", k=c.KVCH),
                                    ckv_v[:, :, qi, :])
                            nc.scalar.dma_start(
                                gin_ckv.ap()[:, c.KVCH * 128:CKR, :]
                                .rearrange("q p x -> p q x"),
                                krope_ch[:].rearrange(
                                    "p (q x) -> p q x", q=NQT))
                            nc.gpsimd.collective_compute(
                                "AllGather", mybir.AluOpType.bypass,
                                replica_groups=RG,
                                ins=[gin_ckv.ap().opt()],
                                outs=[gout_ckv.ap().opt()])
                # big rope tables are only needed by phase C q-rope
                nc.sync.dma_start(cos_sb[:], cos_d.ap())
                nc.sync.dma_start(sin_sb[:], sin_d.ap())
                with tc.high_priority():
                    a_norm(ssq, c.QR, qa_ch)
                    qa_v = qa_all[:].rearrange(
                        "p (k q x) -> p k q x", k=c.QRCH, q=NQT)
                    for qi in range(NQT - 1, -1, -1):
                        nc.scalar.dma_start(
                            gin_qa.ap()[qi].rearrange("k p x -> p k x"),
                            qa_v[:, :, qi, :])
                        nc.gpsimd.collective_compute(
                            "AllGather", mybir.AluOpType.bypass,
                            replica_groups=RG,
                            ins=[gin_qa.ap()[qi].opt()],
                            outs=[gout_qa.ap()[qi].opt()])

            # ---------------- phase B: kv b-projection -----------------
            with tc.tile_pool(name="pB", bufs=2) as pB, \
                 tc.tile_pool(name="pB_ev", bufs=4) as pBe, \
                 tc.tile_pool(name="pB_ps", bufs=3, space="PSUM") as psB:
                wkb_sb = []
                wv_sb = []
                for kc in range(c.KVCH):
                    t = pB.tile([128, c.HPC * c.NOPE], BF16, name=f"wkb_{kc}",
                                bufs=1)
                    nc.sync.dma_start(t[:], wkb_d.ap()[kc])
                    wkb_sb.append(t)
                    t = pB.tile([128, c.HPC * c.VD], BF16, name=f"wv_{kc}",
                                bufs=1)
                    nc.sync.dma_start(t[:], wv_d.ap()[kc])
                    wv_sb.append(t)
                # scheduler hint: B follows the ckv gather; keep it from
                # being interleaved before the phase-A norm/gather ops
                tc.tile_set_cur_wait(1)
                for j in range(NQT):
                    # gathered block j: cols [512j, 512j+512), 64-col runs
                    # interleaved across cores -> natural column order
                    ckv_T = []
                    for kc in range(c.KVCH):
                        t = pB.tile([128, 512], BF16, name=f"ckvT_{kc}")
                        nc.sync.dma_start(
                            t[:].rearrange("p (c x) -> p c x", c=N_CORES),
                            gout_ckv.ap()[:, j, kc * 128:(kc + 1) * 128]
                            .rearrange("c p x -> p c x"))
                        ckv_T.append(t)
                    for par in range(2):
                        nc.sync.dma_start(
                            krope2[par][64 * par:64 * par + 64,
                                        512 * j:512 * (j + 1)].rearrange(
                                "p (c x) -> p c x", c=N_CORES),
                            gout_ckv.ap()[:, j, c.KVCH * 128:CKR]
                            .rearrange("c p x -> p c x"))
                    for m in range(c.HPC):
                        ps = psB.tile([128, 512], F32, name="psB")
                        for kc in range(c.KVCH):
                            nc.tensor.matmul(
                                ps[:], wkb_sb[kc][:, m * 128:(m + 1) * 128],
                                ckv_T[kc][:], start=(kc == 0),
                                stop=(kc == c.KVCH - 1))
                        nc.vector.tensor_copy(
                            knopeT[m][:, 512 * j:512 * (j + 1)], ps[:])
                    for kk in range(4):
                        ki = 4 * j + kk
                        ps = psB.tile([128, c.HPC * c.VD], F32, name="psB")
                        for kc in range(c.KVCH):
                            nc.tensor.matmul(
                                ps[:], ckv_T[kc][:, kk * 128:(kk + 1) * 128],
                                wv_sb[kc][:], start=(kc == 0),
                                stop=(kc == c.KVCH - 1))
                        nc.vector.tensor_copy(v_sb[ki][:], ps[:])

            # ---------------- phase C: q, attention, out-proj ----------
            with tc.tile_pool(name="pC", bufs=1) as pC, \
                 tc.tile_pool(name="pC2", bufs=2) as pC2, \
                 tc.tile_pool(name="pCe", bufs=2) as pCe, \
                 tc.tile_pool(name="pCo", bufs=6) as pCo, \
                 tc.tile_pool(name="pCx", bufs=6) as pCx, \
                 tc.tile_pool(name="pC_mm", bufs=2, space="PSUM") as psM, \
                 tc.tile_pool(name="pC_sT", bufs=3, space="PSUM") as psT, \
                 tc.tile_pool(name="pC_oT", bufs=2, space="PSUM") as psO, \
                 tc.tile_pool(name="pC_den", bufs=1, space="PSUM") as psD:
                wqb_sb = []
                for k in range(c.QRCH):
                    t = pC.tile([128, c.HPC * c.QD], BF16, name=f"wqb_{k}")
                    nc.sync.dma_start(t[:], wqb_d.ap()[k])
                    wqb_sb.append(t)
                wo_sb = []
                for k in range(c.HPC):
                    t = pC.tile([128, c.D], BF16, name=f"wo_{k}")
                    nc.sync.dma_start(t[:], wo_d.ap()[k])
                    wo_sb.append(t)
                mask_all = pC.tile([128, 4 * 512], BF16, name="mask_all")
                nc.sync.dma_start(mask_all[:], mask_d.ap())
                mask_sb = [mask_all[:, j * 512:(j + 1) * 512]
                           for j in range(4)]

                for qi in range(NQT - 1, -1, -1):
                    # scheduler hint: C tiles run in reverse qi order after B
                    tc.tile_set_cur_wait(2 + (NQT - 1 - qi))
                    q0 = qi * 512
                    qa_sb = []
                    for k in range(c.QRCH):
                        t = pC2.tile([128, 512], BF16, name=f"qa_{k}")
                        nc.sync.dma_start(
                            t[:].rearrange("p (c x) -> p c x", c=N_CORES),
                            gout_qa.ap()[qi, :, k].rearrange("c p x -> p c x"))
                        qa_sb.append(t)

                    qnopeT = [pC2.tile([128, 512], BF16, name=f"qnopeT_{m}")
                              for m in range(c.HPC)]
                    qrope_ch = [pC2.tile([128, 512], BF16, name=f"qrope_{j}")
                                for j in range(NROPE_CH)]
                    for m in range(QB_M):
                        ps = psM.tile([128, 512], F32, name="psm")
                        for k in range(c.QRCH):
                            nc.tensor.matmul(
                                ps[:], wqb_sb[k][:, m * 128:(m + 1) * 128],
                                qa_sb[k][:], start=(k == 0),
                                stop=(k == c.QRCH - 1))
                        if m < c.HPC:
                            nc.vector.tensor_copy(qnopeT[m][:], ps[:])
                        else:
                            ro = pCe.tile([128, 512], F32R, name="ro")
                            nc.vector.tensor_copy(ro[:], ps[:])
                            rps = psM.tile([128, 512], F32, name="psm")
                            nc.tensor.matmul(rps[:], rot_sb[:], ro[:])
                            rk = pCe.tile([128, 512], F32, name="qrk")
                            nc.vector.tensor_copy(rk[:], rps[:])
                            a = pCe.tile([128, 512], F32, name="qra")
                            b = pCe.tile([128, 512], F32, name="qrb")
                            nc.vector.tensor_mul(
                                a[:], ro[:], cos_sb[:, q0:q0 + 512])
                            nc.vector.tensor_mul(
                                b[:], rk[:], sin_sb[:, q0:q0 + 512])
                            nc.vector.tensor_add(qrope_ch[m - c.HPC][:],
                                                 a[:], b[:])

                    oT_sb = [pC2.tile([128, 512], BF16, name=f"oT_{h}")
                             for h in range(c.HPC)]
                    nki = 4 * (qi + 1)
                    for h in range(c.HPC):
                        oT_ps = psO.tile([128, 512], F32, name="psO")
                        den_ps = psD.tile([1, 512], F32, name="psD")
                        for ki in range(nki):
                            sT_ps = psT.tile([128, 512], F32, name="psT")
                            nc.tensor.matmul(
                                sT_ps[:],
                                knopeT[h][:, ki * 128:(ki + 1) * 128],
                                qnopeT[h][:], start=True, stop=False)
                            nc.tensor.matmul(
                                sT_ps[:],
                                krope2[h % 2][:, ki * 128:(ki + 1) * 128],
                                qrope_ch[h // 2][:], start=False, stop=True)
                            ex = pCx.tile([128, 512], BF16, name="expT")
                            nc.scalar.activation(ex[:], sT_ps[:], AF.Exp)
                            jj = ki - (nki - 4)
                            if jj >= 0:
                                nc.vector.tensor_mul(ex[:], ex[:],
                                                     mask_sb[jj])
                            nc.tensor.matmul(den_ps[:], ones_col_b[:], ex[:],
                                             start=(ki == 0),
                                             stop=(ki == nki - 1))
                            nc.tensor.matmul(
                                oT_ps[:], v_sb[ki][:, h * c.VD:(h + 1) * c.VD],
                                ex[:], start=(ki == 0), stop=(ki == nki - 1))
                        rec = pCe.tile([1, 512], F32R, name="rec")
                        with nc.allow_low_precision(reason="fp32r for bcast"):
                            nc.vector.reciprocal(rec[:], den_ps[:])
                        bc_sb = pCe.tile([128, 512], F32R, name="bc_sb")
                        nc.gpsimd.partition_broadcast(bc_sb[:], rec[:])
                        nc.vector.tensor_mul(oT_sb[h][:], oT_ps[:], bc_sb[:])

                    for m in range(c.DCH):
                        ps = psM.tile([128, 512], F32, name="psm")
                        for k in range(c.HPC):
                            nc.tensor.matmul(
                                ps[:], wo_sb[k][:, m * 128:(m + 1) * 128],
                                oT_sb[k][:], start=(k == 0),
                                stop=(k == c.HPC - 1))
                        ob = pCo.tile([128, 512], F32, name="ob")
                        nc.vector.tensor_copy(ob[:], ps[:])
                        nc.scalar.dma_start(
                            out_d.ap()[m * 128:(m + 1) * 128, q0:q0 + 512],
                            ob[:])
    nc.compile()
    return nc


# --------------------------------------------------------------------------
# public entry point
# --------------------------------------------------------------------------

_CACHED = {}


def _get_nc(cfg):
    key = cfg
    if key not in _CACHED:
        _CACHED[key] = build(cfg)
    return _CACHED[key]


def kernel(hidden_states, Wq_a, q_a_ln_w, Wq_b, Wkv_a, kv_a_ln_w, Wkv_b, Wo):
    cfg = FULL
    in_maps = prep_inputs(cfg, hidden_states, Wq_a, q_a_ln_w, Wq_b, Wkv_a,
                          kv_a_ln_w, Wkv_b, Wo)
    nc = _get_nc(cfg)
    res = run_bass_kernel_spmd(nc, in_maps, core_ids=list(range(N_CORES)))
    acc = np.zeros((cfg.D, cfg.S), np.float32)
    for r in res.results:
        acc += r["outT"]
    return np.ascontiguousarray(acc.T).reshape(1, cfg.S, cfg.D)


# revision 4
# speedup vs baseline: 1.0624x; 1.0472x over previous
"""DeepSeek-V3 MLA forward (B=1, S=2048, D=4096, H=32) on 8 TRN2 NeuronCores.

v2: tensor-parallel over heads (4 heads/core) for the b-projections /
attention / out-proj, sequence-parallel for the low-rank a-projections.
Each core computes the a-projections (q_a, compressed kv, shared rope key)
for 4 strips of 64 columns (one strip per 512-wide query tile), normalizes
them locally (rms scales, ln weights folded into the b-weights host-side),
and the strips are exchanged with device AllGathers: one early gather for
ckv+krope (feeds the kv b-projection), then one qa gather per query tile
issued in reverse tile order so attention for the largest (last) causal
block starts as early as possible. The post-out-proj all-reduce stays
host-side (sum of per-core partials).

Layouts follow v1: activations feature-major (x^T), scores transposed so
softmax reduction runs on the PE via ones-matmuls, P@V consumes exp^T
directly. All matmul operands bf16 (f32r for rope rotation / broadcast
helpers), accumulation fp32 in PSUM. The attention inner loop is software
pipelined (den/P@V matmuls trail their exp by two key blocks), softmax
reciprocal broadcast runs on gpsimd, and a short warmup matmul burst gets
the PE to full clock before the first projection chain.
"""

import math
from dataclasses import dataclass

import ml_dtypes
import numpy as np

import concourse.bass as bass
import concourse.bass_isa as bass_isa
import concourse.mybir as mybir
import concourse.tile as tile
from concourse import bacc
from concourse.bass_utils import run_bass_kernel_spmd

F32 = mybir.dt.float32
F32R = mybir.dt.float32r
BF16 = mybir.dt.bfloat16
AF = mybir.ActivationFunctionType
BF16NP = ml_dtypes.bfloat16

N_CORES = 8
EPS = 1e-6
THETA = 10000.0


@dataclass(frozen=True)
class Cfg:
    S: int = 2048
    D: int = 4096
    QR: int = 1536      # q lora rank
    KVR: int = 512      # kv lora rank
    H: int = 32         # total heads
    HPC: int = 4        # heads per core
    NOPE: int = 128
    ROPE: int = 64
    VD: int = 128
    STRIP: int = 64     # per-core columns per query tile

    @property
    def QD(self):
        return self.NOPE + self.ROPE

    @property
    def DCH(self):
        return self.D // 128

    @property
    def QRCH(self):
        return self.QR // 128

    @property
    def KVCH(self):
        return self.KVR // 128

    @property
    def AM(self):          # a-proj m-chunks: q rank + kv rank + 1 rope(64pad)
        return self.QRCH + self.KVCH + 1

    @property
    def NQT(self):         # 512-wide query tiles
        return self.S // 512

    @property
    def NKI(self):         # 128-wide key blocks
        return self.S // 128

    @property
    def LC(self):          # local a-proj columns (one strip per query tile)
        return self.NQT * self.STRIP


FULL = Cfg()


# --------------------------------------------------------------------------
# host-side input preparation
# --------------------------------------------------------------------------

def _rope_perm(rope):
    # deepseek interleave: xp = concat(x[0::2], x[1::2]) acting on rope dims
    return np.concatenate([np.arange(0, rope, 2), np.arange(1, rope, 2)])


def prep_inputs(cfg, hidden_states, Wq_a, q_a_ln_w, Wq_b, Wkv_a, kv_a_ln_w,
                Wkv_b, Wo):
    c = cfg
    hs = np.asarray(hidden_states, np.float32).reshape(c.S, c.D)
    Wq_a = np.asarray(Wq_a, np.float32)
    Wq_b = np.asarray(Wq_b, np.float32)
    Wkv_a = np.asarray(Wkv_a, np.float32)
    Wkv_b = np.asarray(Wkv_b, np.float32)
    Wo = np.asarray(Wo, np.float32)
    q_a_ln_w = np.asarray(q_a_ln_w, np.float32)
    kv_a_ln_w = np.asarray(kv_a_ln_w, np.float32)

    hT = np.ascontiguousarray(hs.T)                      # [D, S]

    # combined a-proj weight, padded to AM*128 cols, ckv chunks FIRST so the
    # ckv+krope gather can start before the (3x larger) q chunks compute.
    # Layout [AM, 128, DCH*128] so each m-chunk loads with one contiguous DMA.
    perm_a = _rope_perm(c.ROPE)
    wa = np.concatenate(
        [Wkv_a[:, :c.KVR],                               # ckv chunks 0..3
         Wkv_a[:, c.KVR:][:, perm_a],                    # rope (64)
         np.zeros((c.D, 128 - c.ROPE), np.float32),      # pad rope chunk
         Wq_a], axis=1)                                  # [D, AM*128]
    wa = wa.reshape(c.DCH, 128, c.AM, 128)               # [dch, p, m, c]
    wa = np.ascontiguousarray(wa.transpose(2, 1, 0, 3))  # [m, p, dch, c]
    wa = wa.reshape(c.AM, 128, c.D)

    # per-head-group b-projections / out-proj
    qd, nope, rope, vd = c.QD, c.NOPE, c.ROPE, c.VD
    scale = qd ** (-0.5)
    wqb_all = (Wq_b * q_a_ln_w[:, None]).reshape(c.QR, c.H, qd) * scale
    perm = _rope_perm(rope)
    wqb_nope = wqb_all[:, :, :nope]
    wqb_rope = wqb_all[:, :, nope:][:, :, perm]
    wkv_all = (Wkv_b * kv_a_ln_w[:, None]).reshape(c.KVR, c.H, nope + vd)

    # rotary tables, feature-major, replicated to 128 rows
    inv_freq = 1.0 / (THETA ** (np.arange(0, rope, 2, np.float32) / rope))
    freqs = np.outer(np.arange(c.S, dtype=np.float32), inv_freq)  # [S, 32]
    cosT = np.tile(np.cos(freqs).T, (4, 1)).astype(np.float32)    # [128, S]
    sinT = np.tile(np.sin(freqs).T, (4, 1)).astype(np.float32)
    # rotate-half as a PE matmul: rot = R @ x, R block-diag over two 64-row
    # rope groups, R = [[0, -I32], [I32, 0]] per group. lhsT = R.T.
    R = np.zeros((128, 128), np.float32)
    for blk in (0, 64):
        for i in range(32):
            R[blk + i, blk + i + 32] = -1.0
            R[blk + i + 32, blk + i] = 1.0
    rotT = np.ascontiguousarray(R.T)

    # diagonal-tile masks: mask01[j][r, q] = 1 if 128*j + r <= q
    j = np.arange(4)[:, None, None]
    r = np.arange(128)[None, :, None]
    q = np.arange(512)[None, None, :]
    mask01 = ((128 * j + r) <= q).astype(BF16NP)
    mask01 = np.ascontiguousarray(
        mask01.transpose(1, 0, 2)).reshape(128, 4 * 512)

    shared = {
        "wa": wa.astype(BF16NP),
        "cosT": cosT.astype(BF16NP),
        "sinT": sinT.astype(BF16NP),
        "rotT": rotT,
        "ones_f": np.ones((128, 128), np.float32),
        "mask01": mask01,
    }
    in_maps = []
    for core in range(N_CORES):
        # local columns: strip `core` of each query tile
        cols = np.concatenate(
            [512 * qi + 64 * core + np.arange(64) for qi in range(c.NQT)])
        # [128, DCH*LC] so the whole local slice loads as one contiguous DMA
        hT_loc = np.ascontiguousarray(
            hT[:, cols].reshape(c.DCH, 128, c.LC).transpose(1, 0, 2)
        ).reshape(128, c.DCH * c.LC)
        cos_loc = cosT[0:64, cols]
        sin_loc = sinT[0:64, cols]

        hsel = np.arange(core * c.HPC, (core + 1) * c.HPC)
        wqb_c = np.concatenate(
            [wqb_nope[:, hsel].reshape(c.QR, c.HPC * nope),
             wqb_rope[:, hsel].reshape(c.QR, c.HPC * rope)], axis=1)
        wqb_c = np.ascontiguousarray(
            wqb_c.reshape(c.QRCH, 128, c.HPC * qd)).astype(BF16NP)
        wkb_c = np.ascontiguousarray(
            wkv_all[:, hsel, :nope].reshape(c.KVCH, 128, c.HPC * nope)
        ).astype(BF16NP)
        wv_c = np.ascontiguousarray(
            wkv_all[:, hsel, nope:].reshape(c.KVCH, 128, c.HPC * vd)
        ).astype(BF16NP)
        wo_c = np.ascontiguousarray(
            Wo.reshape(c.H, vd, c.D)[hsel]).astype(BF16NP)
        in_maps.append(dict(
            shared,
            hT=hT_loc.astype(BF16NP),
            cos_loc=np.ascontiguousarray(cos_loc).astype(BF16NP),
            sin_loc=np.ascontiguousarray(sin_loc).astype(BF16NP),
            wqb=wqb_c, wkb=wkb_c, wv=wv_c, wo=wo_c))
    return in_maps


# --------------------------------------------------------------------------
# kernel builder
# --------------------------------------------------------------------------

def build(cfg):
    c = cfg
    nc = bacc.Bacc("TRN2", target_bir_lowering=False, debug=False,
                   num_devices=N_CORES)

    hT_d = nc.declare_dram_parameter("hT", [128, c.DCH * c.LC], BF16, isOutput=False)
    wa_d = nc.declare_dram_parameter("wa", [c.AM, 128, c.D], BF16, isOutput=False)
    wqb_d = nc.declare_dram_parameter("wqb", [c.QRCH, 128, c.HPC * c.QD], BF16, isOutput=False)
    wkb_d = nc.declare_dram_parameter("wkb", [c.KVCH, 128, c.HPC * c.NOPE], BF16, isOutput=False)
    wv_d = nc.declare_dram_parameter("wv", [c.KVCH, 128, c.HPC * c.VD], BF16, isOutput=False)
    wo_d = nc.declare_dram_parameter("wo", [c.HPC, 128, c.D], BF16, isOutput=False)
    cos_d = nc.declare_dram_parameter("cosT", [128, c.S], BF16, isOutput=False)
    sin_d = nc.declare_dram_parameter("sinT", [128, c.S], BF16, isOutput=False)
    cosl_d = nc.declare_dram_parameter("cos_loc", [64, c.LC], BF16, isOutput=False)
    sinl_d = nc.declare_dram_parameter("sin_loc", [64, c.LC], BF16, isOutput=False)
    rot_d = nc.declare_dram_parameter("rotT", [128, 128], F32R, isOutput=False)
    ones_d = nc.declare_dram_parameter("ones_f", [128, 128], F32R, isOutput=False)
    mask_d = nc.declare_dram_parameter("mask01", [128, 4 * 512], BF16, isOutput=False)
    out_d = nc.declare_dram_parameter("outT", [c.D, c.S], F32, isOutput=True)

    NQT = c.NQT
    CKCH = c.KVCH + 1                 # ckv chunks + krope chunk
    CKR = c.KVCH * 128 + 64           # gathered ckv rows (krope not padded)
    # gather bounce buffers (collectives need DRAM operands; out Shared)
    gin_ckv = nc.dram_tensor("gin_ckv", [NQT, CKR, 64], BF16)
    gout_ckv = nc.dram_tensor("gout_ckv", [N_CORES, NQT, CKR, 64], BF16,
                              addr_space="Shared")
    gin_qa = nc.dram_tensor("gin_qa", [NQT, c.QRCH, 128, 64], BF16)
    gout_qa = nc.dram_tensor("gout_qa", [NQT, N_CORES, c.QRCH, 128, 64], BF16,
                             addr_space="Shared")

    RG = [list(range(N_CORES))]
    NROPE_CH = c.HPC * c.ROPE // 128          # rope m-chunks in wqb (2)
    QB_M = c.HPC + NROPE_CH                   # 6

    with tile.TileContext(nc) as tc:
        with tc.tile_pool(name="persist", bufs=1) as pp:
            # persistent tiles
            cos_sb = pp.tile([128, c.S], BF16, name="cos_sb")
            sin_sb = pp.tile([128, c.S], BF16, name="sin_sb")
            rot_sb = pp.tile([128, 128], F32R, name="rot_sb")
            ones_sb = pp.tile([128, 128], F32R, name="ones_sb")
            nc.sync.dma_start(rot_sb[:], rot_d.ap())
            nc.sync.dma_start(ones_sb[:], ones_d.ap())
            ones_col_f = ones_sb[:, 0:1]
            ones_row_f = ones_sb[0:1, :]
            ones_col_b = pp.tile([128, 1], BF16, name="ones_col_b")
            ones_row_b = pp.tile([1, 128], BF16, name="ones_row_b")
            nc.vector.memset(ones_col_b[:], 1.0)
            nc.vector.memset(ones_row_b[:], 1.0)

            # B/C shared residents
            knopeT = [pp.tile([128, c.S], BF16, name=f"knopeT_{m}")
                      for m in range(c.HPC)]
            v_sb = [pp.tile([128, c.HPC * c.VD], BF16, name=f"v_sb_{ki}")
                    for ki in range(c.NKI)]
            krope2 = [pp.tile([128, c.S], BF16, name=f"krope2_{par}")
                      for par in range(2)]
            nc.vector.memset(krope2[0][:], 0.0)
            nc.vector.memset(krope2[1][:], 0.0)

            # ---------------- phase A: sharded a-projections -----------
            with tc.tile_pool(name="pA", bufs=1) as pA, \
                 tc.tile_pool(name="pA_w", bufs=3) as pAw, \
                 tc.tile_pool(name="pA_ev", bufs=4) as pAe, \
                 tc.tile_pool(name="pA_ps", bufs=2, space="PSUM") as psA, \
                 tc.tile_pool(name="pA_ps1", bufs=1, space="PSUM") as psA1:
                # first a-proj weight chunk + first hT quarter load before
                # anything else so the PE can start right away
                wa_pre = []
                hT_all = pA.tile([128, c.DCH * c.LC], BF16, name="hT_all")
                w4 = c.DCH * c.LC // 4
                t = pAw.tile([128, c.D], BF16, name="wa_sb")
                nc.sync.dma_start(t[:], wa_d.ap()[0])
                wa_pre.append(t)
                nc.sync.dma_start(hT_all[:, 0:w4], hT_d.ap()[:, 0:w4])
                t = pAw.tile([128, c.D], BF16, name="wa_sb")
                nc.sync.dma_start(t[:], wa_d.ap()[1])
                wa_pre.append(t)
                for q in range(1, 4):
                    nc.sync.dma_start(hT_all[:, q * w4:(q + 1) * w4],
                                      hT_d.ap()[:, q * w4:(q + 1) * w4])
                cosl_sb = pA.tile([64, c.LC], BF16, name="cosl_sb")
                sinl_sb = pA.tile([64, c.LC], BF16, name="sinl_sb")
                nc.sync.dma_start(cosl_sb[:], cosl_d.ap())
                nc.sync.dma_start(sinl_sb[:], sinl_d.ap())
                hT_sb = [hT_all[:, k * c.LC:(k + 1) * c.LC]
                         for k in range(c.DCH)]
                # PE pstate warmup: ~4us of throwaway matmuls on the ones
                # tile so the first real chains run at full clock
                wps = psA1.tile([128, 128], F32, name="warm_ps")
                for _ in range(10):
                    nc.tensor.matmul(wps[:], ones_sb[:], ones_sb[:])

                ckv_all = pA.tile([128, c.KVCH * c.LC], BF16, name="ckv_all")
                ckv_ch = [ckv_all[:, i * c.LC:(i + 1) * c.LC]
                          for i in range(c.KVCH)]
                krope_ch = pA.tile([64, c.LC], BF16, name="krope_ch")
                qa_all = pA.tile([128, c.QRCH * c.LC], BF16, name="qa_all")
                qa_ch = [qa_all[:, i * c.LC:(i + 1) * c.LC]
                         for i in range(c.QRCH)]
                ssq = psA1.tile([1, c.LC], F32, name="ssq_q")
                ssc = psA1.tile([1, c.LC], F32, name="ssq_c")

                def a_norm(ps1, denom, chunks):
                    """rs = rsqrt(mean+eps) of ps1; chunks *= broadcast(rs)."""
                    t = pAe.tile([1, c.LC], F32, name="rms_t")
                    nc.vector.tensor_scalar(
                        t[:], ps1[:], 1.0 / denom, EPS,
                        mybir.AluOpType.mult, mybir.AluOpType.add)
                    st = pAe.tile([1, c.LC], F32, name="rms_st")
                    nc.scalar.activation(st[:], t[:], AF.Sqrt)
                    rc = pAe.tile([1, c.LC], F32R, name="rms_rc")
                    with nc.allow_low_precision(reason="fp32r for PE bcast"):
                        nc.vector.reciprocal(rc[:], st[:])
                    bc_ps = psA.tile([128, c.LC], F32, name="bc_ps", bufs=1)
                    nc.tensor.matmul(bc_ps[:], ones_row_f, rc[:])
                    bc_sb = pAe.tile([128, c.LC], F32, name="bc_sb")
                    nc.vector.tensor_copy(bc_sb[:], bc_ps[:])
                    for ch in chunks:
                        nc.vector.tensor_mul(ch, ch, bc_sb[:])

                # m order: ckv chunks, krope, then qa chunks
                for m in range(c.AM):
                    if m < 2:
                        wa_sb = wa_pre[m]
                    else:
                        wa_sb = pAw.tile([128, c.D], BF16, name="wa_sb")
                        nc.sync.dma_start(wa_sb[:], wa_d.ap()[m])
                    ps = psA.tile([128, c.LC], F32, name="psA")
                    for k in range(c.DCH):
                        nc.tensor.matmul(
                            ps[:], wa_sb[:, k * 128:(k + 1) * 128],
                            hT_sb[k], start=(k == 0), stop=(k == c.DCH - 1))
                    if m < c.KVCH:
                        nc.vector.tensor_copy(ckv_ch[m], ps[:])
                        x2 = pAe.tile([128, c.LC], F32R, name="x2")
                        nc.vector.tensor_mul(x2[:], ckv_ch[m], ckv_ch[m])
                        nc.tensor.matmul(ssc[:], ones_col_f, x2[:],
                                         start=(m == 0), stop=(m == c.KVCH - 1))
                    elif m == c.KVCH:
                        # shared rope key: rows 0:64 of this chunk
                        kr = pAe.tile([64, c.LC], F32R, name="kr")
                        nc.vector.tensor_copy(kr[:], ps[0:64, :])
                        rps = psA.tile([64, c.LC], F32, name="rot_ps", bufs=1)
                        nc.tensor.matmul(rps[:], rot_sb[0:64, 0:64], kr[:])
                        rk = pAe.tile([64, c.LC], F32, name="rk")
                        nc.vector.tensor_copy(rk[:], rps[:])
                        a = pAe.tile([64, c.LC], F32, name="ra")
                        b = pAe.tile([64, c.LC], F32, name="rb")
                        nc.vector.tensor_mul(a[:], kr[:], cosl_sb[:])
                        nc.vector.tensor_mul(b[:], rk[:], sinl_sb[:])
                        nc.vector.tensor_add(krope_ch[:], a[:], b[:])
                    else:
                        mq = m - CKCH
                        nc.vector.tensor_copy(qa_ch[mq], ps[:])
                        x2 = pAe.tile([128, c.LC], F32R, name="x2")
                        nc.vector.tensor_mul(x2[:], qa_ch[mq], qa_ch[mq])
                        nc.tensor.matmul(ssq[:], ones_col_f, x2[:],
                                         start=(mq == 0),
                                         stop=(mq == c.QRCH - 1))
                    if m == c.KVCH - 1:
                        with tc.high_priority():
                            a_norm(ssc, c.KVR, ckv_ch)
                    if m == c.KVCH:
                        # ship ckv + krope strips, gather early
                        with tc.high_priority():
                            ckv_v = ckv_all[:].rearrange(
                                "p (k q x) -> p k q x", k=c.KVCH, q=NQT)
                            for qi in range(NQT):
                                nc.scalar.dma_start(
                                    gin_ckv.ap()[qi, 0:c.KVCH * 128]
                                    .rearrange("(k p) x -> p k x", k=c.KVCH),
                                    ckv_v[:, :, qi, :])
                            nc.scalar.dma_start(
                                gin_ckv.ap()[:, c.KVCH * 128:CKR, :]
                                .rearrange("q p x -> p q x"),
                                krope_ch[:].rearrange(
                                    "p (q x) -> p q x", q=NQT))
                            nc.gpsimd.collective_compute(
                                "AllGather", mybir.AluOpType.bypass,
                                replica_groups=RG,
                                ins=[gin_ckv.ap().opt()],
                                outs=[gout_ckv.ap().opt()])
                # big rope tables are only needed by phase C q-rope
                nc.sync.dma_start(cos_sb[:], cos_d.ap())
                nc.sync.dma_start(sin_sb[:], sin_d.ap())
                with tc.high_priority():
                    a_norm(ssq, c.QR, qa_ch)
                    qa_v = qa_all[:].rearrange(
                        "p (k q x) -> p k q x", k=c.QRCH, q=NQT)
                    for qi in range(NQT - 1, -1, -1):
                        nc.scalar.dma_start(
                            gin_qa.ap()[qi].rearrange("k p x -> p k x"),
                            qa_v[:, :, qi, :])
                        nc.gpsimd.collective_compute(
                            "AllGather", mybir.AluOpType.bypass,
                            replica_groups=RG,
                            ins=[gin_qa.ap()[qi].opt()],
                            outs=[gout_qa.ap()[qi].opt()])

            # ---------------- phase B: kv b-projection -----------------
            with tc.tile_pool(name="pB", bufs=2) as pB, \
                 tc.tile_pool(name="pB_ev", bufs=4) as pBe, \
                 tc.tile_pool(name="pB_ps", bufs=3, space="PSUM") as psB:
                wkb_sb = []
                wv_sb = []
                for kc in range(c.KVCH):
                    t = pB.tile([128, c.HPC * c.NOPE], BF16, name=f"wkb_{kc}",
                                bufs=1)
                    nc.sync.dma_start(t[:], wkb_d.ap()[kc])
                    wkb_sb.append(t)
                    t = pB.tile([128, c.HPC * c.VD], BF16, name=f"wv_{kc}",
                                bufs=1)
                    nc.sync.dma_start(t[:], wv_d.ap()[kc])
                    wv_sb.append(t)
                # scheduler hint: B follows the ckv gather; keep it from
                # being interleaved before the phase-A norm/gather ops
                tc.tile_set_cur_wait(1)
                for j in range(NQT):
                    # gathered block j: cols [512j, 512j+512), 64-col runs
                    # interleaved across cores -> natural column order
                    ckv_T = []
                    for kc in range(c.KVCH):
                        t = pB.tile([128, 512], BF16, name=f"ckvT_{kc}")
                        nc.sync.dma_start(
                            t[:].rearrange("p (c x) -> p c x", c=N_CORES),
                            gout_ckv.ap()[:, j, kc * 128:(kc + 1) * 128]
                            .rearrange("c p x -> p c x"))
                        ckv_T.append(t)
                    for par in range(2):
                        nc.sync.dma_start(
                            krope2[par][64 * par:64 * par + 64,
                                        512 * j:512 * (j + 1)].rearrange(
                                "p (c x) -> p c x", c=N_CORES),
                            gout_ckv.ap()[:, j, c.KVCH * 128:CKR]
                            .rearrange("c p x -> p c x"))
                    for m in range(c.HPC):
                        ps = psB.tile([128, 512], F32, name="psB")
                        for kc in range(c.KVCH):
                            nc.tensor.matmul(
                                ps[:], wkb_sb[kc][:, m * 128:(m + 1) * 128],
                                ckv_T[kc][:], start=(kc == 0),
                                stop=(kc == c.KVCH - 1))
                        nc.vector.tensor_copy(
                            knopeT[m][:, 512 * j:512 * (j + 1)], ps[:])
                    for kk in range(4):
                        ki = 4 * j + kk
                        ps = psB.tile([128, c.HPC * c.VD], F32, name="psB")
                        for kc in range(c.KVCH):
                            nc.tensor.matmul(
                                ps[:], ckv_T[kc][:, kk * 128:(kk + 1) * 128],
                                wv_sb[kc][:], start=(kc == 0),
                                stop=(kc == c.KVCH - 1))
                        nc.vector.tensor_copy(v_sb[ki][:], ps[:])

            # ---------------- phase C: q, attention, out-proj ----------
            with tc.tile_pool(name="pC", bufs=1) as pC, \
                 tc.tile_pool(name="pC2", bufs=2) as pC2, \
                 tc.tile_pool(name="pCe", bufs=2) as pCe, \
                 tc.tile_pool(name="pCo", bufs=6) as pCo, \
                 tc.tile_pool(name="pCx", bufs=6) as pCx, \
                 tc.tile_pool(name="pC_mm", bufs=2, space="PSUM") as psM, \
                 tc.tile_pool(name="pC_sT", bufs=3, space="PSUM") as psT, \
                 tc.tile_pool(name="pC_oT", bufs=2, space="PSUM") as psO, \
                 tc.tile_pool(name="pC_den", bufs=1, space="PSUM") as psD:
                wqb_sb = []
                for k in range(c.QRCH):
                    t = pC.tile([128, c.HPC * c.QD], BF16, name=f"wqb_{k}")
                    nc.sync.dma_start(t[:], wqb_d.ap()[k])
                    wqb_sb.append(t)
                wo_sb = []
                for k in range(c.HPC):
                    t = pC.tile([128, c.D], BF16, name=f"wo_{k}")
                    nc.sync.dma_start(t[:], wo_d.ap()[k])
                    wo_sb.append(t)
                mask_all = pC.tile([128, 4 * 512], BF16, name="mask_all")
                nc.sync.dma_start(mask_all[:], mask_d.ap())
                mask_sb = [mask_all[:, j * 512:(j + 1) * 512]
                           for j in range(4)]

                for qi in range(NQT - 1, -1, -1):
                    # scheduler hint: C tiles run in reverse qi order after B
                    tc.tile_set_cur_wait(2 + (NQT - 1 - qi))
                    q0 = qi * 512
                    qa_sb = []
                    for k in range(c.QRCH):
                        t = pC2.tile([128, 512], BF16, name=f"qa_{k}")
                        nc.sync.dma_start(
                            t[:].rearrange("p (c x) -> p c x", c=N_CORES),
                            gout_qa.ap()[qi, :, k].rearrange("c p x -> p c x"))
                        qa_sb.append(t)

                    qnopeT = [pC2.tile([128, 512], BF16, name=f"qnopeT_{m}")
                              for m in range(c.HPC)]
                    qrope_ch = [pC2.tile([128, 512], BF16, name=f"qrope_{j}")
                                for j in range(NROPE_CH)]
                    for m in range(QB_M):
                        ps = psM.tile([128, 512], F32, name="psm")
                        for k in range(c.QRCH):
                            nc.tensor.matmul(
                                ps[:], wqb_sb[k][:, m * 128:(m + 1) * 128],
                                qa_sb[k][:], start=(k == 0),
                                stop=(k == c.QRCH - 1))
                        if m < c.HPC:
                            nc.vector.tensor_copy(qnopeT[m][:], ps[:])
                        else:
                            ro = pCe.tile([128, 512], F32R, name="ro")
                            nc.vector.tensor_copy(ro[:], ps[:])
                            rps = psM.tile([128, 512], F32, name="psm")
                            nc.tensor.matmul(rps[:], rot_sb[:], ro[:])
                            rk = pCe.tile([128, 512], F32, name="qrk")
                            nc.vector.tensor_copy(rk[:], rps[:])
                            a = pCe.tile([128, 512], F32, name="qra")
                            b = pCe.tile([128, 512], F32, name="qrb")
                            nc.vector.tensor_mul(
                                a[:], ro[:], cos_sb[:, q0:q0 + 512])
                            nc.vector.tensor_mul(
                                b[:], rk[:], sin_sb[:, q0:q0 + 512])
                            nc.vector.tensor_add(qrope_ch[m - c.HPC][:],
                                                 a[:], b[:])

                    oT_sb = [pC2.tile([128, 512], BF16, name=f"oT_{h}")
                             for h in range(c.HPC)]
                    nki = 4 * (qi + 1)
                    for h in range(c.HPC):
                        oT_ps = psO.tile([128, 512], F32, name="psO")
                        den_ps = psD.tile([1, 512], F32, name="psD")
                        exs = []

                        def consume(ki):
                            ex = exs[ki]
                            nc.tensor.matmul(den_ps[:], ones_col_b[:], ex[:],
                                             start=(ki == 0),
                                             stop=(ki == nki - 1))
                            nc.tensor.matmul(
                                oT_ps[:], v_sb[ki][:, h * c.VD:(h + 1) * c.VD],
                                ex[:], start=(ki == 0), stop=(ki == nki - 1))

                        for ki in range(nki):
                            sT_ps = psT.tile([128, 512], F32, name="psT")
                            nc.tensor.matmul(
                                sT_ps[:],
                                knopeT[h][:, ki * 128:(ki + 1) * 128],
                                qnopeT[h][:], start=True, stop=False)
                            nc.tensor.matmul(
                                sT_ps[:],
                                krope2[h % 2][:, ki * 128:(ki + 1) * 128],
                                qrope_ch[h // 2][:], start=False, stop=True)
                            ex = pCx.tile([128, 512], BF16, name="expT")
                            nc.scalar.activation(ex[:], sT_ps[:], AF.Exp)
                            jj = ki - (nki - 4)
                            if jj >= 0:
                                nc.vector.tensor_mul(ex[:], ex[:],
                                                     mask_sb[jj])
                            exs.append(ex)
                            # den/oT run two ki behind their exp, so the PE
                            # never waits out the ACT latency
                            if ki >= 2:
                                consume(ki - 2)
                        consume(nki - 2)
                        consume(nki - 1)
                        rec = pCe.tile([1, 512], F32R, name="rec")
                        with nc.allow_low_precision(reason="fp32r for bcast"):
                            nc.vector.reciprocal(rec[:], den_ps[:])
                        bc_sb = pCe.tile([128, 512], F32R, name="bc_sb")
                        nc.gpsimd.partition_broadcast(bc_sb[:], rec[:])
                        nc.vector.tensor_mul(oT_sb[h][:], oT_ps[:], bc_sb[:])

                    for m in range(c.DCH):
                        ps = psM.tile([128, 512], F32, name="psm")
                        for k in range(c.HPC):
                            nc.tensor.matmul(
                                ps[:], wo_sb[k][:, m * 128:(m + 1) * 128],
                                oT_sb[k][:], start=(k == 0),
                                stop=(k == c.HPC - 1))
                        ob = pCo.tile([128, 512], F32, name="ob")
                        nc.vector.tensor_copy(ob[:], ps[:])
                        nc.scalar.dma_start(
                            out_d.ap()[m * 128:(m + 1) * 128, q0:q0 + 512],
                            ob[:])
    nc.compile()
    return nc


# --------------------------------------------------------------------------
# public entry point
# --------------------------------------------------------------------------

_CACHED = {}


def _get_nc(cfg):
    key = cfg
    if key not in _CACHED:
        _CACHED[key] = build(cfg)
    return _CACHED[key]


def kernel(hidden_states, Wq_a, q_a_ln_w, Wq_b, Wkv_a, kv_a_ln_w, Wkv_b, Wo):
    cfg = FULL
    in_maps = prep_inputs(cfg, hidden_states, Wq_a, q_a_ln_w, Wq_b, Wkv_a,
                          kv_a_ln_w, Wkv_b, Wo)
    nc = _get_nc(cfg)
    res = run_bass_kernel_spmd(nc, in_maps, core_ids=list(range(N_CORES)))
    acc = np.zeros((cfg.D, cfg.S), np.float32)
    for r in res.results:
        acc += r["outT"]
    return np.ascontiguousarray(acc.T).reshape(1, cfg.S, cfg.D)
